# revision 1
# baseline (speedup 1.0000x reference)
"""DeepIRT (DKVMN) Trainium2 kernel.

Contract: kernel(**inputs) takes the FULL unsharded inputs of reference.py's
setup_inputs() and returns the full [64, 500] float32 output.

Strategy (8 NeuronCores, pure data parallel over batch):
  - each core handles BL=8 batch rows; tables/weights replicated.
  - precompute per core: gather k/v embeddings (indirect DMA), transpose to
    [d, token] layout with PE, compute w = softmax(k Mk^T), e = sigmoid(v We+be),
    a = tanh(v Wa+ba); pack per-step PE coefficient streams into DRAM:
      wd9[t]  = [9, 400]: row 0 = ones(400); row 1+b = w[b,t,:] placed in a
                block-diagonal at columns [b*50, b*50+50)
      lg9[t]  = [9, 128]: row 0 = ones(128); row 1+b = -e[b,t,:]
      la9[t]  = [9, 128]: row 0 = zeros;     row 1+b =  a[b,t,:]
  - recurrence over t (state Mv layout [128(d), 8b*50m] in SBUF):
      G  = lg9[t]^T @ wd9[t]  -> PSUM [128, 400] = 1 - w (x) e   (PE)
      WA = la9[t]^T @ wd9[t]  -> PSUM [128, 400] =      w (x) a  (PE)
      X  = Mv * G                                   (DVE tensor_tensor)
      Mv = X + WA                                   (DVE tensor_tensor)
      RX = reduce_m(X)          [128, 8]            (DVE tensor_reduce)
      read_t = (S_prev - RX) * (1/e_t)  (since reduce_m(Mv*(w e)) = e * read)
      S      = RX + a_t                 (since softmax rows sum to 1)
    1/e is computed exactly as 1 + exp(-z) from the sigmoid logits z.
  - final: f = tanh([reads, k] Wf + bf) via two accumulating matmuls on the
    [d, token] archives; stu/qd heads via [128,1] matmuls into an [8, 500]
    PSUM tile; predict = sigmoid(3*stu - qd) -> output [8, 500].

Tokens are ordered b-major: tok = b*T_PAD + t with T padded to T_PAD=512
(4096 tokens) so every 128-token tile is full and lies within one batch row;
padded slots use index 0 and are never read by the recurrence or the output
path.
"""

import numpy as np

import concourse.bass as bass
import concourse.bacc as bacc
import concourse.tile as tile
import concourse.mybir as mybir
from concourse.masks import make_identity

F32 = mybir.dt.float32
BF16 = mybir.dt.bfloat16
I32 = mybir.dt.int32
I16 = mybir.dt.int16
OP = mybir.AluOpType
AF = mybir.ActivationFunctionType

NUM_CONCEPT = 1000
D = 128
M = 50
B_FULL, T = 64, 500
NCORES = 8
BL = B_FULL // NCORES          # 8 batch rows per core
T_PAD = 512
NTOK = T_PAD * BL              # 4096 padded tokens, b-major: tok = b*T_PAD + t
NCH = NTOK // 128              # 32 gather/transpose chunks
W9 = 9 * 400                   # wd9 row stride in elements
L9 = 9 * 128
A9 = 9 * 128
STEP_CHUNK = 16                # recurrence steps loaded per DMA




class _Shist:
    """Ring of the last few S tiles; index by absolute step (t = -1 is init)."""

    def __init__(self, s_init, depth=8):
        self.buf = [None] * depth
        self.depth = depth
        self.buf[(-1) % depth] = s_init
        self.n = 0

    def append(self, tile):
        self.buf[self.n % self.depth] = tile
        self.n += 1

    def __getitem__(self, t):
        # t in [-1, ...): S_hist[t] = S after step t; d1(tl) uses [tl-1],[tl]
        return self.buf[t % self.depth]


def _ap(t, offset, dims):
    return bass.AP(t.tensor, offset, [list(d) for d in dims])


def build_program(debug_taps=False, phases=(1, 2, 3, 4)):
    nc = bacc.Bacc("TRN2", target_bir_lowering=False, debug=False)

    # ---------------- I/O ----------------
    h = {}
    h["concept_seq"] = nc.declare_dram_parameter("concept_seq", [BL, T], I32, isOutput=False)
    h["correct_seq"] = nc.declare_dram_parameter("correct_seq", [BL, T], I32, isOutput=False)
    h["embed_key"] = nc.declare_dram_parameter("embed_key", [NUM_CONCEPT, D], F32, isOutput=False)
    h["embed_value"] = nc.declare_dram_parameter("embed_value", [2 * NUM_CONCEPT, D], F32, isOutput=False)
    h["Mk"] = nc.declare_dram_parameter("Mk", [M, D], F32, isOutput=False)
    h["Mv0"] = nc.declare_dram_parameter("Mv0", [M, D], F32, isOutput=False)
    h["We"] = nc.declare_dram_parameter("We", [D, D], F32, isOutput=False)
    h["be"] = nc.declare_dram_parameter("be", [D], F32, isOutput=False)
    h["Wa"] = nc.declare_dram_parameter("Wa", [D, D], F32, isOutput=False)
    h["ba"] = nc.declare_dram_parameter("ba", [D], F32, isOutput=False)
    h["Wf"] = nc.declare_dram_parameter("Wf", [2 * D, D], F32, isOutput=False)
    h["bf"] = nc.declare_dram_parameter("bf", [D], F32, isOutput=False)
    h["Wab"] = nc.declare_dram_parameter("Wab", [D, 1], F32, isOutput=False)
    h["bab"] = nc.declare_dram_parameter("bab", [1], F32, isOutput=False)
    h["Wd"] = nc.declare_dram_parameter("Wd", [D, 1], F32, isOutput=False)
    h["bd"] = nc.declare_dram_parameter("bd", [1], F32, isOutput=False)
    out_h = nc.declare_dram_parameter("out", [BL, T], F32, isOutput=True)
    dbg = {}
    if debug_taps:
        for n in ("dbg_eT", "dbg_aT", "dbg_erecip"):
            dbg[n] = nc.declare_dram_parameter(n, [128, NTOK], F32, isOutput=True)
        for n in ("dbg_kT", "dbg_reads", "dbg_fT"):
            dbg[n] = nc.declare_dram_parameter(n, [128, NTOK], BF16, isOutput=True)
        dbg["dbg_state"] = nc.declare_dram_parameter("dbg_state", [128, BL * M], F32, isOutput=True)
        dbg["dbg_prob"] = nc.declare_dram_parameter("dbg_prob", [1, NTOK], F32, isOutput=True)
        dbg["dbg_wd9"] = nc.declare_dram_parameter("dbg_wd9", [9, T_PAD, 400], BF16, isOutput=True)
        dbg["dbg_lg9"] = nc.declare_dram_parameter("dbg_lg9", [9, T_PAD, 128], BF16, isOutput=True)
        dbg["dbg_la9"] = nc.declare_dram_parameter("dbg_la9", [9, T_PAD, 128], BF16, isOutput=True)

    with tile.TileContext(nc) as tc:
        _emit(nc, tc, h, out_h, dbg, phases)
    nc.finalize()
    return nc


def _emit(nc, tc, h, out_h, dbg=None, phases=(1, 2, 3, 4)):
    from contextlib import ExitStack

    ctx = ExitStack()
    with ctx:
        # ---- pools ----
        persist = ctx.enter_context(tc.tile_pool(name="persist", bufs=1))
        dram = ctx.enter_context(tc.tile_pool(name="dram", bufs=1, space="DRAM"))

        # DRAM coefficient streams, row-major by PE contraction row so the
        # recurrence chunk loads are 9 contiguous runs; split into 4
        # t-quarters so the recurrence can start after the first quarter
        # of phase 2 (tile-granular dependency tracking).
        wdq = [dram.tile([9, 128, 400], BF16, name=f"wdq{j}") for j in range(4)]
        lgq = [dram.tile([9, 128, 128], BF16, name=f"lgq{j}") for j in range(4)]
        laq = [dram.tile([9, 128, 128], BF16, name=f"laq{j}") for j in range(4)]

        # persistent SBUF archives ([d, token] layouts, t-major tokens)
        k_T = persist.tile([128, NTOK], BF16)
        v_T = persist.tile([128, NTOK], BF16)
        e_T = persist.tile([128, NTOK], F32)
        erecip_T = persist.tile([128, NTOK], F32)
        a_T = persist.tile([128, NTOK], F32)
        reads_T = persist.tile([128, NTOK], BF16)
        f_T = persist.tile([128, NTOK], BF16)

        # small persistent constants / weights
        ident = persist.tile([128, 128], F32)
        make_identity(nc, ident)
        ident_neg = persist.tile([128, 128], F32)
        nc.vector.tensor_scalar(out=ident_neg, in0=ident, scalar1=-1.0,
                                scalar2=None, op0=OP.mult)
        ones128 = persist.tile([128, 128], BF16)
        nc.vector.memset(ones128, 1.0)
        ones400 = persist.tile([128, 400], BF16)
        nc.vector.memset(ones400, 1.0)
        zeros1600 = persist.tile([128, 1600], BF16)
        nc.vector.memset(zeros1600, 0.0)
        zeros400f = persist.tile([1, 400], F32)
        nc.vector.memset(zeros400f, 0.0)

        We_f32 = persist.tile([128, 128], F32)
        nc.sync.dma_start(out=We_f32, in_=h["We"][:, :])
        We_sb = persist.tile([128, 128], BF16)
        nc.vector.tensor_copy(out=We_sb, in_=We_f32)
        Wa_f32 = persist.tile([128, 128], F32)
        nc.sync.dma_start(out=Wa_f32, in_=h["Wa"][:, :])
        Wa_sb = persist.tile([128, 128], BF16)
        nc.vector.tensor_copy(out=Wa_sb, in_=Wa_f32)
        Wf_r32 = persist.tile([128, 128], F32)
        nc.sync.dma_start(out=Wf_r32, in_=h["Wf"][0:128, :])
        Wf_r = persist.tile([128, 128], BF16)
        nc.vector.tensor_copy(out=Wf_r, in_=Wf_r32)
        Wf_k32 = persist.tile([128, 128], F32)
        nc.sync.dma_start(out=Wf_k32, in_=h["Wf"][128:256, :])
        Wf_k = persist.tile([128, 128], BF16)
        nc.vector.tensor_copy(out=Wf_k, in_=Wf_k32)
        Wab_sb = persist.tile([128, 1], F32)
        nc.sync.dma_start(out=Wab_sb, in_=h["Wab"][:, :])
        Wd_sb = persist.tile([128, 1], F32)
        nc.sync.dma_start(out=Wd_sb, in_=h["Wd"][:, :])
        Mk_sb = persist.tile([50, 128], F32)
        nc.sync.dma_start(out=Mk_sb, in_=h["Mk"][:, :])
        Mv0_sb = persist.tile([50, 128], F32)
        nc.sync.dma_start(out=Mv0_sb, in_=h["Mv0"][:, :])

        def col(name, n=128):
            t = persist.tile([n, 1], F32, name=name)
            nc.sync.dma_start(out=t, in_=_ap(h[name[:-4]][:], 0, [[1, n], [1, 1]]))
            return t

        be_col = col("be_col")
        ba_col = col("ba_col")
        bf_col = col("bf_col")
        neg_be_col = persist.tile([128, 1], F32)
        nc.vector.tensor_scalar(out=neg_be_col, in0=be_col, scalar1=-1.0, scalar2=None, op0=OP.mult)
        neg_ba_col = persist.tile([128, 1], F32)
        nc.vector.tensor_scalar(out=neg_ba_col, in0=ba_col, scalar1=-1.0, scalar2=None, op0=OP.mult)

        # ---- fill DRAM streams: zeros + ones rows (per quarter) ----
        QW = 128 * 400   # wd quarter row stride (elems per r)
        QL = 128 * 128
        wdqf = [t.rearrange("r t c -> (r t c)") for t in wdq]
        lgqf = [t.rearrange("r t c -> (r t c)") for t in lgq]
        laqf = [t.rearrange("r t c -> (r t c)") for t in laq]
        fillqs = (nc.sync, nc.gpsimd, nc.scalar)
        fq = 0
        for j in range(4):
            # wd row 0 <- ones
            fillqs[fq % 3].dma_start(
                out=_ap(wdqf[j][:], 0, [[400, 128], [1, 400]]),
                in_=ones400); fq += 1
            # wd rows 1-8 <- zeros (2 x [128,1600] covers 8*51200 elems)
            for i in range(2):
                fillqs[fq % 3].dma_start(
                    out=_ap(wdqf[j][:], QW + i * 128 * 1600, [[1600, 128], [1, 1600]]),
                    in_=zeros1600); fq += 1
            # lg row 0 <- ones
            fillqs[fq % 3].dma_start(
                out=_ap(lgqf[j][:], 0, [[128, 128], [1, 128]]),
                in_=ones128); fq += 1
            # la row 0 <- zeros
            fillqs[fq % 3].dma_start(
                out=_ap(laqf[j][:], 0, [[128, 128], [1, 128]]),
                in_=zeros1600[:, 0:128]); fq += 1

        # ---- transpose Mv0 and Mk once; Mv0^T kept in SBUF for state init ----
        Mv0T_sb = persist.tile([128, 50], F32)
        MkT_sb = persist.tile([128, 50], BF16)
        with tc.tile_pool(name="init_ps", bufs=1, space="PSUM") as initp:
            mv0t = initp.tile([128, 50], F32)
            nc.tensor.transpose(mv0t, Mv0_sb, ident[0:50, 0:50])
            nc.any.tensor_copy(out=Mv0T_sb, in_=mv0t)
            mkt_ps = initp.tile([128, 50], F32)
            nc.tensor.transpose(mkt_ps, Mk_sb, ident[0:50, 0:50])
            nc.any.tensor_copy(out=MkT_sb, in_=mkt_ps)

        # =========== phase 1: indices, on-chip table gathers ===========
        if 1 not in phases:
            return
        # Instead of 64 row-gather DMAs + 64 PE transposes, transpose the two
        # embedding tables once into SBUF ([d, row] layout) and use the GPSIMD
        # ap_gather to produce k_T / v_T directly in [d, token] layout.
        idxk_dram = dram.tile([NTOK], I16)
        idxv_dram = dram.tile([NTOK], I16)
        with tc.tile_pool(name="ph1", bufs=1) as ph1, \
             tc.tile_pool(name="ph1t", bufs=3) as ph1t, \
             tc.tile_pool(name="ph1ps", bufs=3, space="PSUM") as ph1ps:

            cseq = ph1.tile([8, T_PAD], I32)
            crse = ph1.tile([8, T_PAD], I32)
            nc.vector.memset(cseq, 0)
            nc.vector.memset(crse, 0)
            nc.sync.dma_start(out=cseq[:, 0:T], in_=h["concept_seq"][:, :])
            nc.gpsimd.dma_start(out=crse[:, 0:T], in_=h["correct_seq"][:, :])

            # x = concept + 1000*correct (exact in fp32, cast back to ints)
            cseq_f = ph1.tile([8, T_PAD], F32)
            nc.vector.tensor_copy(out=cseq_f, in_=cseq)
            crse_f = ph1.tile([8, T_PAD], F32)
            nc.vector.tensor_copy(out=crse_f, in_=crse)
            x_f = ph1.tile([8, T_PAD], F32)
            nc.vector.scalar_tensor_tensor(out=x_f, in0=crse_f, scalar=float(NUM_CONCEPT),
                                           in1=cseq_f, op0=OP.mult, op1=OP.add)
            ck16s = ph1.tile([8, T_PAD], I16)
            nc.vector.tensor_copy(out=ck16s, in_=cseq)
            xv16s = ph1.tile([8, T_PAD], I16)
            nc.vector.tensor_copy(out=xv16s, in_=x_f)

            # bounce through DRAM to rewrap indices: tok i -> (partition i%16,
            # col i//16), replicated into each 16-partition group
            G16 = NTOK // 16
            ck16 = ph1.tile([128, G16], I16)
            xv16 = ph1.tile([128, G16], I16)
            for srct, drt, dstt, eng in ((ck16s, idxk_dram, ck16, nc.sync),
                                         (xv16s, idxv_dram, xv16, nc.gpsimd)):
                eng.dma_start(out=_ap(drt[:], 0, [[T_PAD, 8], [1, T_PAD]]), in_=srct)
                for j in range(8):
                    eng.dma_start(out=dstt[16 * j:16 * (j + 1), :],
                                  in_=_ap(drt[:], 0, [[1, 16], [16, G16]]))

            # transpose the tables into SBUF: [128(d), rows]
            ekt = ph1.tile([128, NUM_CONCEPT], F32)
            evt = ph1.tile([128, 2 * NUM_CONCEPT], F32)
            gi = 0
            for tbl, dst_t, nrows in ((h["embed_key"], ekt, NUM_CONCEPT),
                                      (h["embed_value"], evt, 2 * NUM_CONCEPT)):
                for g0 in range(0, nrows, 512):
                    gn = min(512, nrows - g0)          # rows in this 4-group
                    nq = (gn + 127) // 128             # 128-row blocks
                    rows4 = ph1t.tile([128, 512], F32, tag="rows4")
                    eng = (nc.sync, nc.gpsimd)[gi % 2]
                    gi += 1
                    # one DMA for up to 4 blocks: dst (p, q, c), src row q*128+p
                    full = gn // 128
                    if full:
                        eng.dma_start(
                            out=rows4[:, 0:full * 128].rearrange("p (q c) -> p q c", c=128),
                            in_=_ap(tbl[:], g0 * 128,
                                    [[128, 128], [128 * 128, full], [1, 128]]))
                    if gn % 128:
                        rem = gn % 128
                        eng.dma_start(
                            out=rows4[0:rem, full * 128:(full + 1) * 128],
                            in_=tbl[g0 + full * 128:g0 + gn, :])
                    for q in range(nq):
                        n = min(128, gn - q * 128)
                        tps = ph1ps.tile([128, 128], F32, tag="tps")
                        nc.tensor.transpose(tps[:, 0:n],
                                            rows4[0:n, q * 128:(q + 1) * 128],
                                            ident[0:n, 0:n])
                        nc.any.tensor_copy(out=dst_t[:, g0 + q * 128:g0 + q * 128 + n],
                                           in_=tps[:, 0:n])

            # v first: the e/a activation pass is the longest chain consumer
            vgat = ph1.tile([128, NTOK], F32)
            nc.gpsimd.ap_gather(
                out_ap=vgat.rearrange("p (n d) -> p n d", d=1),
                in_ap=evt.rearrange("p (n d) -> p n d", d=1),
                idxs_ap=xv16, channels=128, num_elems=2 * NUM_CONCEPT, d=1, num_idxs=NTOK)
            nc.vector.tensor_copy(out=v_T, in_=vgat)
            kgat = ph1.tile([128, NTOK], F32)
            nc.gpsimd.ap_gather(
                out_ap=kgat.rearrange("p (n d) -> p n d", d=1),
                in_ap=ekt.rearrange("p (n d) -> p n d", d=1),
                idxs_ap=ck16, channels=128, num_elems=NUM_CONCEPT, d=1, num_idxs=NTOK)
            nc.vector.tensor_copy(out=k_T, in_=kgat)

        # =========== phase 2: e, a, 1/e, w; pack DRAM streams ===========
        if 2 not in phases:
            return
        rec_ld = ctx.enter_context(tc.tile_pool(name="rec_ld", bufs=2))
        with tc.tile_pool(name="ph2", bufs=3) as ph2, \
             tc.tile_pool(name="ph2ps", bufs=1, space="PSUM") as ph2ps:

            # grouped by activation function to avoid ACT table reloads:
            # all sigmoids first (e), then 1/e via DVE reciprocal (no Exp
            # pass), then all tanhs (a)
            for c in range(8):
                sl = slice(c * 512, (c + 1) * 512)
                elog = ph2ps.tile([128, 512], F32, tag="elog", bufs=2)
                nc.tensor.matmul(elog, We_sb, v_T[:, sl], start=True, stop=True)
                nc.scalar.activation(out=e_T[:, sl], in_=elog, func=AF.Sigmoid, bias=be_col)
            for c in range(8):
                sl = slice(c * 512, (c + 1) * 512)
                alog = ph2ps.tile([128, 512], F32, tag="alog", bufs=2)
                nc.tensor.matmul(alog, Wa_sb, v_T[:, sl], start=True, stop=True)
                nc.scalar.activation(out=a_T[:, sl], in_=alog, func=AF.Tanh,
                     bias=neg_ba_col, scale=-1.0)

            # w softmax + scatter; q-major so quarter 0 completes first and
            # the recurrence can start. Per quarter, accumulate all 8 batches
            # in SBUF and emit ONE DMA per stream (12 DMAs total).
            qs = (nc.sync, nc.gpsimd)
            for q in range(4):
                w8 = ph2.tile([128, 8, 50], BF16, tag="w8", bufs=2)
                ea8 = ph2.tile([128, 8, 256], BF16, tag="ea8", bufs=2)
                for b in range(BL):
                    p = 4 * b + q
                    sl = slice(p * 128, (p + 1) * 128)
                    wlog = ph2ps.tile([128, 50], F32, tag="wlog", bufs=2)
                    nc.tensor.matmul(wlog, k_T[:, sl], MkT_sb, start=True, stop=True)
                    # logits are O(1): skip the softmax max-subtraction
                    wexp = ph2.tile([128, 50], F32, tag="wexp")
                    sumexp = ph2.tile([128, 1], F32, tag="sumexp")
                    nc.scalar.activation(out=wexp, in_=wlog, func=AF.Exp,
                                         accum_out=sumexp)
                    rsum = ph2.tile([128, 1], F32, tag="rsum")
                    nc.vector.reciprocal(out=rsum, in_=sumexp)
                    nc.vector.tensor_scalar(out=w8[:, b, :], in0=wexp, scalar1=rsum,
                                            scalar2=-1.0, op0=OP.mult, op1=OP.mult)
                    ea = ph2ps.tile([128, 256], F32, tag="ea", bufs=2)
                    nc.tensor.transpose(ea[:, 0:128], e_T[:, sl], ident)
                    nc.tensor.transpose(ea[:, 128:256], a_T[:, sl], ident)
                    nc.vector.tensor_copy(out=ea8[:, b, :], in_=ea)
                # dst elem (p', b, c): wd row 1+b, t=p', col b*50+c.
                # quarter 0 goes on SP so nothing head-blocks the recurrence's
                # first chunk loads; later quarters use the idle Pool queue.
                sq = nc.sync if q == 0 else nc.gpsimd
                sq.dma_start(
                    out=_ap(wdqf[q][:], QW, [[400, 128], [QW + 50, 8], [1, 50]]),
                    in_=w8)
                sq.dma_start(
                    out=_ap(lgqf[q][:], QL, [[128, 128], [QL, 8], [1, 128]]),
                    in_=ea8[:, :, 0:128])
                sq.dma_start(
                    out=_ap(laqf[q][:], QL, [[128, 128], [QL, 8], [1, 128]]),
                    in_=ea8[:, :, 128:256])

            # 1/e deferred here: only phase 3's read ops consume it, so it
            # must not compete with the scatter chain for the DVE
            for c in range(8):
                sl = slice(c * 512, (c + 1) * 512)
                nc.vector.reciprocal(out=erecip_T[:, sl], in_=e_T[:, sl])

        # =========== phase 3: the 500-step recurrence ===========
        if 3 not in phases:
            return
        # v3: the serial state chain lives entirely on the Pool engine
        # (same-engine dependent ops chain at pure exec rate, ~333ns per
        # [128,400] op; cross-engine sem hops would cost ~200ns each).
        #   PE   : g  = lg9^t wd9 -> gw[t%2][:,0]   (off-chain, 2 banks)
        #   PE   : wa = la9^t wd9 -> gw[t%2][:,1]
        #   Pool : X = st[(t-1)%4] * g              (X single slot: dead
        #   Pool : st[t%4] = X + wa                  after the add)
        #   DVE  : S_t = reduce_m(st[t%4]); read_t = (S_{t-1}-S_t+a_t)*1/e_t
        # PSUM: big tile = 4 banks (4 state slots of 400 + X at tail),
        # gw0/gw1 = 2 banks each -> exactly 8 banks.
        state_fin = persist.tile([128, BL * M], F32)
        with tc.tile_pool(name="rec_sm", bufs=4) as rec_sm, \
             tc.tile_pool(name="rec_ps", bufs=1, space="PSUM") as rec_ps:

            stP = [rec_sm.tile([128, 2, 400], F32, name=f"stP{i}") for i in range(2)]
            X = rec_sm.tile([128, 400], F32, name="Xbuf")
            # two PSUM quads (4 banks each): slots (g0, wa0, g1, wa1) per
            # step-pair; ACT evacuates a whole quad to SBUF bf16 in one op
            # (GPSIMD cannot access PSUM on real hardware).
            gw = [rec_ps.tile([128, 4, 512], F32, name=f"gw{i}") for i in range(2)]

            # init state lives in stP[1] slot 1 (= slot of t=-1)
            for b in range(BL):
                nc.scalar.copy(out=stP[1][:, 1, b * M:(b + 1) * M], in_=Mv0T_sb)
            S3_prev = rec_sm.tile([128, 3, BL], F32, tag="S3", bufs=4)
            nc.vector.tensor_reduce(
                out=S3_prev[:, 2, :],
                in_=stP[1][:, 1, :].rearrange("p (b m) -> p b m", b=BL),
                axis=mybir.AxisListType.X, op=OP.add)

            er_tb = erecip_T.rearrange("p (b t) -> p t b", t=T_PAD)
            rd_tb = reads_T.rearrange("p (b t) -> p t b", t=T_PAD)
            a_tb = a_T.rearrange("p (b t) -> p t b", t=T_PAD)

            # first two chunks are small (4, 12) with loads spread over 3
            # queues, so the recurrence starts ~6us earlier; the rest are 16.
            chunk_plan = [(0, 4), (4, 12)] + [(j0, min(STEP_CHUNK, T - j0))
                                             for j0 in range(16, T, STEP_CHUNK)]
            for j0, n in chunk_plan:
                qj, jo = j0 // 128, j0 % 128
                q3 = (nc.sync, nc.sync, nc.sync)
                wd_c = rec_ld.tile([9, STEP_CHUNK * 400], BF16, tag="wd_c")
                q3[0].dma_start(out=wd_c[:, 0:n * 400],
                                in_=_ap(wdqf[qj][:], jo * 400, [[QW, 9], [1, n * 400]]))
                lg_c = rec_ld.tile([9, STEP_CHUNK * 128], BF16, tag="lg_c")
                q3[1].dma_start(out=lg_c[:, 0:n * 128],
                                in_=_ap(lgqf[qj][:], jo * 128, [[QL, 9], [1, n * 128]]))
                la_c = rec_ld.tile([9, STEP_CHUNK * 128], BF16, tag="la_c")
                q3[2].dma_start(out=la_c[:, 0:n * 128],
                                in_=_ap(laqf[qj][:], jo * 128, [[QL, 9], [1, n * 128]]))

                for s0 in range(0, n, 2):
                    # PE: 4 matmuls of the pair into one PSUM quad
                    quad = gw[((j0 + s0) // 2) % 2]
                    for ds in range(2):
                        s = s0 + ds
                        nc.tensor.matmul(quad[:, 2 * ds, 0:400],
                                         lg_c[:, s * 128:(s + 1) * 128],
                                         wd_c[:, s * 400:(s + 1) * 400],
                                         start=True, stop=True)
                        nc.tensor.matmul(quad[:, 2 * ds + 1, 0:400],
                                         la_c[:, s * 128:(s + 1) * 128],
                                         wd_c[:, s * 400:(s + 1) * 400],
                                         start=True, stop=True)
                    gwsb = rec_sm.tile([128, 4, 400], BF16, tag="gwsb", bufs=4)
                    if j0 + s0 < 4:
                        # ACT is still draining phase-2 wexps; DVE's queue
                        # clears ~5us earlier, so the first pairs evac there
                        nc.vector.tensor_copy(out=gwsb, in_=quad[:, :, 0:400])
                    else:
                        # evac split: ACT takes cols [0:372], DVE [372:400]
                        nc.scalar.copy(out=gwsb[:, :, 0:372], in_=quad[:, :, 0:372])
                        nc.vector.tensor_copy(out=gwsb[:, :, 372:400],
                                              in_=quad[:, :, 372:400])
                    pk = (j0 + s0) // 2       # pair index
                    cur = stP[pk % 2]
                    for ds in range(2):
                        t = j0 + s0 + ds
                        tp = t - 1
                        prev = stP[((tp // 2) % 2)][:, tp % 2, :] if tp >= 0 \
                            else stP[1][:, 1, :]
                        nc.gpsimd.tensor_tensor(out=X, in0=prev,
                                                in1=gwsb[:, 2 * ds, :], op=OP.mult)
                        nc.gpsimd.tensor_tensor(out=cur[:, ds, :], in0=X,
                                                in1=gwsb[:, 2 * ds + 1, :], op=OP.add)
                    # pair-batched read extraction on DVE
                    S3 = rec_sm.tile([128, 3, BL], F32, tag="S3", bufs=4)
                    nc.vector.tensor_copy(out=S3[:, 0, :], in_=S3_prev[:, 2, :])
                    nc.vector.tensor_reduce(
                        out=S3[:, 1:3, :],
                        in_=cur.rearrange("p s (b m) -> p s b m", b=BL),
                        axis=mybir.AxisListType.X, op=OP.add)
                    t0p = j0 + s0
                    d1p = rec_sm.tile([128, 2, BL], F32, tag="d1p")
                    nc.vector.tensor_tensor(out=d1p, in0=S3[:, 0:2, :],
                                            in1=S3[:, 1:3, :], op=OP.subtract)
                    d2p = rec_sm.tile([128, 2, BL], F32, tag="d2p")
                    nc.vector.tensor_tensor(out=d2p, in0=d1p,
                                            in1=a_tb[:, t0p:t0p + 2, :], op=OP.subtract)
                    nc.vector.tensor_tensor(out=rd_tb[:, t0p:t0p + 2, :], in0=d2p,
                                            in1=er_tb[:, t0p:t0p + 2, :], op=OP.mult)
                    S3_prev = S3

            if dbg:
                nc.scalar.copy(out=state_fin, in_=stP[(((T - 1) // 2) % 2)][:, (T - 1) % 2, :])

        # =========== phase 4: output heads ===========
        if 4 not in phases:
            return
        # zero the padded t in [500, 512) columns of every batch row
        nc.vector.memset(
            reads_T.rearrange("p (b t) -> p b t", t=T_PAD)[:, :, T:T_PAD], 0.0)
        with tc.tile_pool(name="fin", bufs=2) as fin, \
             tc.tile_pool(name="finps", bufs=2, space="PSUM") as finps:
            for c in range(8):
                sl = slice(c * 512, (c + 1) * 512)
                f_ps = finps.tile([128, 512], F32, tag="f_ps")
                nc.tensor.matmul(f_ps, Wf_r, reads_T[:, sl], start=True, stop=False)
                nc.tensor.matmul(f_ps, Wf_k, k_T[:, sl], start=False, stop=True)
                nc.scalar.activation(out=f_T[:, sl], in_=f_ps, func=AF.Tanh, bias=bf_col)

            # heads: [2, 512] PSUM per chunk = {stu_logit; qd_logit}
            Wab0 = fin.tile([128, 2], BF16, tag="wab0")
            nc.vector.memset(Wab0, 0.0)
            nc.vector.tensor_copy(out=Wab0[:, 0:1], in_=Wab_sb)
            W0d = fin.tile([128, 2], BF16, tag="w0d")
            nc.vector.memset(W0d, 0.0)
            nc.vector.tensor_copy(out=W0d[:, 1:2], in_=Wd_sb)
            comb = fin.tile([2, 1], BF16, tag="comb")
            nc.vector.memset(comb, -1.0)
            nc.vector.memset(comb[0:1, :], 3.0)
            bias2 = fin.tile([2, 1], F32, tag="bias2")
            nc.sync.dma_start(out=bias2[0:1, :], in_=_ap(h["bab"][:], 0, [[1, 1], [1, 1]]))
            nc.sync.dma_start(out=bias2[1:2, :], in_=_ap(h["bd"][:], 0, [[1, 1], [1, 1]]))

            prob_row = fin.tile([1, NTOK], F32, tag="prob_row")
            for c in range(8):
                sl = slice(c * 512, (c + 1) * 512)
                hp = finps.tile([2, 512], F32, tag="hp")
                nc.tensor.matmul(hp, Wab0, f_T[:, sl], start=True, stop=False)
                nc.tensor.matmul(hp, W0d, k_T[:, sl], start=False, stop=True)
                ht = fin.tile([2, 512], BF16, tag="ht")
                nc.scalar.activation(out=ht, in_=hp, func=AF.Tanh, bias=bias2)
                lg_ps = finps.tile([1, 512], F32, tag="lg_ps")
                nc.tensor.matmul(lg_ps, comb, ht, start=True, stop=True)
                nc.scalar.activation(out=prob_row[:, sl], in_=lg_ps, func=AF.Sigmoid)
                # chunk c holds exactly batch c's tokens: emit its output row
                # now so the DMA overlaps the remaining chunks
                nc.sync.dma_start(out=out_h[c:c + 1, :],
                                  in_=prob_row[0:1, c * T_PAD:c * T_PAD + T])
            if dbg:
                for name, tile_ in (("dbg_kT", k_T), ("dbg_eT", e_T), ("dbg_aT", a_T),
                                    ("dbg_erecip", erecip_T), ("dbg_reads", reads_T),
                                    ("dbg_fT", f_T), ("dbg_state", state_fin),
                                    ("dbg_prob", prob_row)):
                    nc.sync.dma_start(out=dbg[name][:, :], in_=tile_)



_NC = None
LAST_RESULT = None


def _get_nc():
    global _NC
    if _NC is None:
        _NC = build_program()
    return _NC


def kernel(**inputs):
    global LAST_RESULT
    from concourse.bass_utils import run_bass_kernel_spmd

    nc = _get_nc()
    names = ["concept_seq", "correct_seq", "embed_key", "embed_value", "Mk", "Mv0",
             "We", "be", "Wa", "ba", "Wf", "bf", "Wab", "bab", "Wd", "bd"]
    full = {k: np.ascontiguousarray(np.asarray(inputs[k])) for k in names}
    in_maps = []
    for i in range(NCORES):
        m = dict(full)
        m["concept_seq"] = np.ascontiguousarray(full["concept_seq"][i * BL:(i + 1) * BL])
        m["correct_seq"] = np.ascontiguousarray(full["correct_seq"][i * BL:(i + 1) * BL])
        in_maps.append(m)
    res = run_bass_kernel_spmd(nc, in_maps, core_ids=list(range(NCORES)))
    LAST_RESULT = res
    return np.concatenate([res.results[i]["out"] for i in range(NCORES)], axis=0)


if __name__ == "__main__":
    nc = build_program()
    print("build ok:", len(nc.m.functions[0].instructions) if hasattr(nc.m.functions[0], "instructions") else "n/a")

